# revision 15
# baseline (speedup 1.0000x reference)
"""Tensor-parallel GQA attention prefill (Llama-style) on one TRN2 chip.

Head-sharded across 8 NeuronCores: core c owns q-heads [4c, 4c+4) and
kv-head c.  x is replicated (pre-transposed on host), wq/wk/wv are
column-sharded, wo row-sharded; each core computes a partial output
[B*S, DIM] and the host sums the 8 partials.

Self-contained: shapes hardcoded for
  x[2,2048,4096] wq[4096,4096] wk/wv[1024,4096] wo[4096,4096]
  32 q heads / 8 kv heads / head_dim 128 / causal prefill (start_pos=0).
"""

import math

import numpy as np
import ml_dtypes

import concourse.bass as bass
import concourse.mybir as mybir
from concourse import bacc
from concourse.tile import TileContext
from concourse.bass_utils import run_bass_kernel_spmd
from concourse.masks import make_identity

BSZ, SEQ, DIM = 2, 2048, 4096
NH, NKV, HD = 32, 8, 128
NCORES = 8
HPC = NH // NCORES          # 4 q heads per core
BS = BSZ * SEQ              # 4096 flattened rows
NJ = BS // 512              # 8 s-chunks of 512
KT = DIM // 128             # 32 contraction tiles
SBLK = 4                    # 512-wide s-blocks per batch
BF16 = mybir.dt.bfloat16
F32 = mybir.dt.float32
NPBF16 = ml_dtypes.bfloat16
ALU = mybir.AluOpType
AF = mybir.ActivationFunctionType


def build_graph():
    nc = bacc.Bacc("TRN2", target_bir_lowering=False)
    xT = nc.declare_dram_parameter("xT", [DIM, BS], BF16, isOutput=False)
    wqT = nc.declare_dram_parameter("wqT", [DIM, HPC * HD], BF16, isOutput=False)
    wkT = nc.declare_dram_parameter("wkT", [DIM, HD], BF16, isOutput=False)
    wvT = nc.declare_dram_parameter("wvT", [DIM, HD], BF16, isOutput=False)
    woT = nc.declare_dram_parameter("woT", [HPC * HD, DIM], BF16, isOutput=False)
    cosT = nc.declare_dram_parameter("cosT", [HD // 2, SEQ], BF16, isOutput=False)
    sinT = nc.declare_dram_parameter("sinT", [HD // 2, SEQ], BF16, isOutput=False)
    tri = nc.declare_dram_parameter("tri", [128, 128], BF16, isOutput=False)
    out = nc.declare_dram_parameter("out", [BS, DIM], BF16, isOutput=True)

    with TileContext(nc) as tc:
        with (
            tc.tile_pool(name="const", bufs=1) as const,
            tc.tile_pool(name="xtp", bufs=2) as xtp,
            tc.tile_pool(name="ropep", bufs=2) as ropep,
            tc.tile_pool(name="ptp", bufs=8) as ptp,
            tc.tile_pool(name="accp", bufs=2) as accp,
            tc.tile_pool(name="recp", bufs=2) as recp,
            tc.tile_pool(name="osb", bufs=3) as osb,
            tc.tile_pool(name="psA", bufs=4, space="PSUM") as psA,
            tc.tile_pool(name="psB", bufs=4, space="PSUM") as psB,
        ):
            # ---- resident constants / weights -------------------------------
            # wq/wk/wv are DMA'd per k-slice inside the j==0 loop so the
            # first matmuls start as soon as their slice lands.
            wq_sb = const.tile([128, KT, HPC * HD], BF16, tag="wq")
            wk_sb = const.tile([128, KT, HD], BF16, tag="wk")
            wv_sb = const.tile([128, KT, HD], BF16, tag="wv")
            cos_sb = const.tile([64, SEQ], BF16, tag="cos")
            nc.sync.dma_start(cos_sb[:], cosT[:])
            sin_sb = const.tile([64, SEQ], BF16, tag="sin")
            nc.sync.dma_start(sin_sb[:], sinT[:])
            tri_sb = const.tile([128, 128], BF16, tag="tri")
            nc.sync.dma_start(tri_sb[:], tri[:])
            # wo is first needed in the attention phase; loaded there.
            wo_sb = const.tile([128, HPC, DIM], BF16, tag="wo")

            ones_sb = const.tile([128, 1], BF16, tag="ones")
            nc.gpsimd.memset(ones_sb[:], 1.0)
            ident = const.tile([128, 128], BF16, tag="ident")
            make_identity(nc, ident[:])

            # Preload the exp table so the first attention exp doesn't pay
            # the ACT_TABLE_LOAD, and run warm-up matmuls on ident during the
            # initial DMA window so HAM unthrottles before the real work.
            scr = const.tile([128, 1], BF16, tag="scr")
            nc.scalar.activation(scr[:], ones_sb[:], AF.Exp)
            warm = psA.tile([128, 512], F32, tag="psA", name="warm")
            for _ in range(130):
                nc.tensor.matmul(warm[:, 0:128], lhsT=ident[:], rhs=ident[:],
                                 start=True, stop=True)

            # ---- resident activations ---------------------------------------
            qT_sb = const.tile([128, HPC, BS], BF16, tag="qT")    # per-head Q^T
            kT_sb = const.tile([128, BS], BF16, tag="kT")         # K^T (d, t)
            v_sb = const.tile([128, BS // 128, HD], BF16, tag="v")  # V (t, d) tiles
            attnT = const.tile([128, HPC, BS], BF16, tag="attnT")  # per-head out^T

            def rope_copy(psum, dst, soff):
                """psum [128,512] (evens-first layout) -> rotated bf16 dst."""
                te = ropep.tile([64, 512], BF16, tag="ropetmpe")
                to = ropep.tile([64, 512], BF16, tag="ropetmpo")
                nc.scalar.copy(te[:], psum[0:64])
                nc.vector.tensor_copy(to[:], psum[64:128])
                cs = cos_sb[:, soff:soff + 512]
                sn = sin_sb[:, soff:soff + 512]
                te = te[:]
                to = to[:]
                t1 = ropep.tile([64, 512], BF16, tag="t1")
                t2 = ropep.tile([64, 512], BF16, tag="t2")
                nc.vector.tensor_tensor(t1[:], te, cs, ALU.mult)
                nc.vector.tensor_tensor(t2[:], to, sn, ALU.mult)
                nc.vector.tensor_tensor(dst[0:64], t1[:], t2[:], ALU.subtract)
                t3 = ropep.tile([64, 512], BF16, tag="t1")
                t4 = ropep.tile([64, 512], BF16, tag="t2")
                nc.vector.tensor_tensor(t3[:], te, sn, ALU.mult)
                nc.vector.tensor_tensor(t4[:], to, cs, ALU.mult)
                nc.vector.tensor_tensor(dst[64:128], t3[:], t4[:], ALU.add)

            # ================= Phase 1: QKV projection =======================
            # single pass over xT per s-chunk: 4 Q accumulators in psA,
            # K/V accumulators in psB.
            for j in range(NJ):
                soff = (j % SBLK) * 512      # within-batch s offset
                js = slice(j * 512, (j + 1) * 512)
                qps = [psA.tile([128, 512], F32, tag="psA", name=f"qps{j}_{c}") for c in range(HPC)]
                kp = psB.tile([128, 512], F32, tag="psB", name=f"kp{j}")
                vp = psB.tile([128, 512], F32, tag="psB", name=f"vp{j}")
                for kc in range(KT // 4):
                    xt = xtp.tile([128, 4, 512], BF16, tag="xt")
                    nc.sync.dma_start(
                        xt[:],
                        xT[kc * 512:(kc + 1) * 512, js].rearrange("(a p) m -> p a m", p=128))
                    if j == 0:
                        for k4 in range(4):
                            ks = slice((kc * 4 + k4) * 128, (kc * 4 + k4 + 1) * 128)
                            nc.sync.dma_start(wq_sb[:, kc * 4 + k4, :], wqT[ks, :])
                            nc.sync.dma_start(wk_sb[:, kc * 4 + k4, :], wkT[ks, :])
                            nc.sync.dma_start(wv_sb[:, kc * 4 + k4, :], wvT[ks, :])
                    for k4 in range(4):
                        k = kc * 4 + k4
                        for c in range(HPC):
                            nc.tensor.matmul(
                                qps[c][:], lhsT=wq_sb[:, k, c * 128:(c + 1) * 128],
                                rhs=xt[:, k4, :], start=(k == 0), stop=(k == KT - 1))
                        nc.tensor.matmul(kp[:], lhsT=wk_sb[:, k, :], rhs=xt[:, k4, :],
                                         start=(k == 0), stop=(k == KT - 1))
                        nc.tensor.matmul(vp[:], lhsT=wv_sb[:, k, :], rhs=xt[:, k4, :],
                                         start=(k == 0), stop=(k == KT - 1))
                # K/V first: attention needs them (and their PSUM slots) at the
                # phase boundary before any Q-rope results.
                rope_copy(kp, kT_sb[:, js], soff)
                # V^T chunk -> natural-layout V tiles via PE transpose.
                # Last chunk's copies go on DVE so the ScalarE queue is clear
                # for the first attention exp right at the phase boundary.
                last = j == NJ - 1
                vtmp = ropep.tile([128, 512], BF16, tag="vtmp")
                if last:
                    nc.vector.tensor_copy(vtmp[:], vp[:])
                else:
                    nc.scalar.copy(vtmp[:], vp[:])
                for sub in range(4):
                    tt = j * 4 + sub
                    pvt = psB.tile([128, 512], BF16, tag="psB", name=f"pvt{j}_{sub}")
                    with nc.allow_low_precision(reason="pure transpose, no accumulation"):
                        nc.tensor.transpose(
                            pvt[:, 0:128], vtmp[:, sub * 128:(sub + 1) * 128], ident[:])
                    if last:
                        nc.vector.tensor_copy(v_sb[:, tt, :], pvt[:, 0:128])
                    else:
                        nc.scalar.copy(v_sb[:, tt, :], pvt[:, 0:128])
                for c in range(HPC):
                    rope_copy(qps[c], qT_sb[:, c, js], soff)
                if j == 4:
                    # mid-phase: DMA bandwidth has headroom here and wo is
                    # needed right after the phase boundary.
                    nc.sync.dma_start(wo_sb[:], woT.rearrange("(a p) m -> p a m", p=128))

            # ================= Phase 2+3: attention + out-proj ===============
            # Out-proj of the previous block is interleaved at attention-tile
            # granularity so the PE never starves while ScalarE runs exp; the
            # den/recip chain of each head is deferred by one head so its
            # den-matmul never blocks the PE FIFO on the DVE accumulation.
            def outproj_group(b, s_lo, s_w, g):
                ngrp = (s_w // 128) * 8
                st, n = g // 8, g % 8
                s0 = b * SEQ + s_lo + st * 128
                po = psA.tile([128, 512], F32, tag="psA", name=f"po{b}_{s_lo}_{g}")
                for dt in range(HPC):
                    nc.tensor.matmul(
                        po[:], lhsT=attnT[:, dt, s0:s0 + 128],
                        rhs=wo_sb[:, dt, n * 512:(n + 1) * 512],
                        start=(dt == 0), stop=(dt == HPC - 1))
                ob = osb.tile([128, 512], BF16, tag="ob")
                if n % 2 == 0:
                    nc.scalar.copy(ob[:], po[:])
                else:
                    nc.vector.tensor_copy(ob[:], po[:])
                nc.sync.dma_start(out[s0:s0 + 128, n * 512:(n + 1) * 512], ob[:])

            def den_chain(h, sg, s_w, ppv, ptsum):
                pden = psB.tile([1, 512], F32, tag="psB", name=f"pden{sg}_{h}")
                nc.tensor.matmul(pden[:, 0:s_w], lhsT=ones_sb[:],
                                 rhs=ptsum[:, 0:s_w], start=True, stop=True)
                recf = recp.tile([1, 512], F32, tag="recf")
                nc.vector.reciprocal_approx_fast(recf[:, 0:s_w], pden[:, 0:s_w])
                rec = recp.tile([1, 512], BF16, tag="rec")
                nc.vector.tensor_copy(rec[:, 0:s_w], recf[:, 0:s_w])
                rep = recp.tile([128, 512], BF16, tag="rep")
                nc.gpsimd.partition_broadcast(rep[:, 0:s_w], rec[:, 0:s_w])
                nc.vector.tensor_tensor(attnT[:, h, sg:sg + s_w],
                                        ppv[:, 0:s_w], rep[:, 0:s_w], ALU.mult)

            # (batch, s_lo, s_width); within a batch s ranges must ascend.
            # The final block is split in two halves so half the last
            # out-proj overlaps the second half's attention.
            blocks = [(0, 0, 512), (0, 512, 512), (0, 1024, 512), (0, 1536, 512),
                      (1, 512, 512), (1, 1024, 512), (1, 1536, 512),
                      (1, 0, 256), (1, 256, 256)]
            prev = None
            for bi, (b, s_lo, s_w) in enumerate(blocks):
                # First block overlaps the tail of the phase-1 rope work on
                # DVE; push its accumulation to the idle GpSimd engine.
                acc_eng = nc.gpsimd if bi == 0 else nc.vector
                sg = b * SEQ + s_lo
                nt = (s_lo + s_w) // 128             # causal t-tiles
                ntile = HPC * nt
                ngrp_prev = (prev[2] // 128) * 8 if prev is not None else 0
                emitted = 0
                tidx = 0
                pend = None
                for h in range(HPC):
                    ppv = psA.tile([128, 512], F32, tag="psA", name=f"ppv{sg}{h}")
                    ptsum = accp.tile([128, 512], BF16, tag="ptsum",
                                      name=f"ptsum{sg}{h}")
                    for ti in range(nt):
                        tg = b * SEQ + ti * 128
                        w0 = max(0, 128 * ti - s_lo)
                        diag = 128 * ti >= s_lo
                        psc = psB.tile([128, 512], F32, tag="psB",
                                       name=f"psc{sg}{h}{ti}")
                        nc.tensor.matmul(psc[:, w0:s_w],
                                         lhsT=kT_sb[:, tg:tg + 128],
                                         rhs=qT_sb[:, h, sg + w0:sg + s_w],
                                         start=True, stop=not diag)
                        if diag:                     # additive causal mask via PE
                            nc.tensor.matmul(psc[:, w0:w0 + 128], lhsT=ident[:],
                                             rhs=tri_sb[:],
                                             start=False, stop=True)
                        if ti == 0:
                            pt = ptsum               # exp seeds the running sum
                        else:
                            pt = ptp.tile([128, 512], BF16, tag="pt")
                        nc.scalar.activation(pt[:, w0:s_w], psc[:, w0:s_w], AF.Exp)
                        if ti > 0:
                            acc_eng.tensor_tensor(ptsum[:, w0:s_w],
                                                  ptsum[:, w0:s_w],
                                                  pt[:, w0:s_w], ALU.add)
                        nc.tensor.matmul(ppv[:, w0:s_w],
                                         lhsT=v_sb[:, tg // 128, :],
                                         rhs=pt[:, w0:s_w], start=(ti == 0),
                                         stop=(ti == nt - 1))
                        tidx += 1
                        if prev is not None:
                            want = tidx * ngrp_prev // ntile
                            while emitted < want:
                                outproj_group(prev[0], prev[1], prev[2], emitted)
                                emitted += 1
                    if pend is not None:
                        den_chain(*pend)
                    pend = (h, sg, s_w, ppv, ptsum)
                if prev is not None:
                    while emitted < ngrp_prev:
                        outproj_group(prev[0], prev[1], prev[2], emitted)
                        emitted += 1
                den_chain(*pend)
                prev = (b, s_lo, s_w)
            for g in range((prev[2] // 128) * 8):
                outproj_group(prev[0], prev[1], prev[2], g)
    nc.finalize()
    return nc


_GRAPH = None


def _get_graph():
    global _GRAPH
    if _GRAPH is None:
        _GRAPH = build_graph()
    return _GRAPH


def prepare_in_maps(x, wq, wk, wv, wo, freqs_cos, freqs_sin, mask, start_pos=0):
    x = np.asarray(x, np.float32)
    wq = np.asarray(wq, np.float32)
    wk = np.asarray(wk, np.float32)
    wv = np.asarray(wv, np.float32)
    wo = np.asarray(wo, np.float32)
    fc = np.asarray(freqs_cos, np.float32)
    fs = np.asarray(freqs_sin, np.float32)

    # evens-first pair permutation (interleaved rope -> rotate-half form)
    perm = np.concatenate([np.arange(0, HD, 2), np.arange(1, HD, 2)])

    def permute_heads(w):
        wr = w.reshape(-1, HD, DIM)[:, perm, :]
        return wr.reshape(-1, DIM)

    wq_p = permute_heads(wq) * (1.0 / math.sqrt(HD))
    wk_p = permute_heads(wk)

    xT = np.ascontiguousarray(x.reshape(BS, DIM).T).astype(NPBF16)
    cosT = np.ascontiguousarray(fc.T).astype(NPBF16)
    sinT = np.ascontiguousarray(fs.T).astype(NPBF16)
    # additive causal triangle for the in-tile diagonal: tri[t, c] = 0 if
    # c >= t else -1e9 (c = column within the 128-wide diagonal strip)
    tt, cc = np.meshgrid(np.arange(128), np.arange(128), indexing="ij")
    tri = np.where(cc >= tt, 0.0, -1e9).astype(NPBF16)

    in_maps = []
    for c in range(NCORES):
        qs = slice(c * HPC * HD, (c + 1) * HPC * HD)
        ks = slice(c * HD, (c + 1) * HD)
        in_maps.append({
            "xT": xT,
            "wqT": np.ascontiguousarray(wq_p[qs, :].T).astype(NPBF16),
            "wkT": np.ascontiguousarray(wk_p[ks, :].T).astype(NPBF16),
            "wvT": np.ascontiguousarray(wv[ks, :].T).astype(NPBF16),
            "woT": np.ascontiguousarray(wo[:, qs].T).astype(NPBF16),
            "cosT": cosT,
            "sinT": sinT,
            "tri": tri,
        })
    return in_maps


def combine_results(results):
    acc = results[0]["out"].astype(np.float64)
    for c in range(1, NCORES):
        acc = acc + results[c]["out"]
    return acc.astype(np.float32).reshape(BSZ, SEQ, DIM)


def run_spmd(in_maps, **kw):
    nc = _get_graph()
    return run_bass_kernel_spmd(nc, in_maps, list(range(NCORES)), **kw)


def kernel(x, wq, wk, wv, wo, freqs_cos, freqs_sin, mask, start_pos=0, **_):
    in_maps = prepare_in_maps(x, wq, wk, wv, wo, freqs_cos, freqs_sin, mask)
    res = run_spmd(in_maps)
    return combine_results(res.results)


# revision 21
# speedup vs baseline: 1.0560x; 1.0560x over previous
"""Tensor-parallel GQA attention prefill (Llama-style) on one TRN2 chip.

Head-sharded across 8 NeuronCores: core c owns q-heads [4c, 4c+4) and
kv-head c.  x is replicated (pre-transposed on host), wq/wk/wv are
column-sharded, wo row-sharded; each core computes a partial output
[B*S, DIM] and the host sums the 8 partials.

Self-contained: shapes hardcoded for
  x[2,2048,4096] wq[4096,4096] wk/wv[1024,4096] wo[4096,4096]
  32 q heads / 8 kv heads / head_dim 128 / causal prefill (start_pos=0).
"""

import math

import numpy as np
import ml_dtypes

import concourse.bass as bass
import concourse.mybir as mybir
from concourse import bacc
from concourse.tile import TileContext
from concourse.bass_utils import run_bass_kernel_spmd
from concourse.masks import make_identity

BSZ, SEQ, DIM = 2, 2048, 4096
NH, NKV, HD = 32, 8, 128
NCORES = 8
HPC = NH // NCORES          # 4 q heads per core
BS = BSZ * SEQ              # 4096 flattened rows
NJ = BS // 512              # 8 s-chunks of 512
KT = DIM // 128             # 32 contraction tiles
SBLK = 4                    # 512-wide s-blocks per batch
BF16 = mybir.dt.bfloat16
F32 = mybir.dt.float32
NPBF16 = ml_dtypes.bfloat16
ALU = mybir.AluOpType
AF = mybir.ActivationFunctionType


def build_graph():
    nc = bacc.Bacc("TRN2", target_bir_lowering=False)
    xT = nc.declare_dram_parameter("xT", [DIM, BS], BF16, isOutput=False)
    wqT = nc.declare_dram_parameter("wqT", [DIM, HPC * HD], BF16, isOutput=False)
    wkT = nc.declare_dram_parameter("wkT", [DIM, HD], BF16, isOutput=False)
    wvT = nc.declare_dram_parameter("wvT", [DIM, HD], BF16, isOutput=False)
    woT = nc.declare_dram_parameter("woT", [HPC * HD, DIM], BF16, isOutput=False)
    cosT = nc.declare_dram_parameter("cosT", [HD // 2, SEQ], BF16, isOutput=False)
    sinT = nc.declare_dram_parameter("sinT", [HD // 2, SEQ], BF16, isOutput=False)
    tri = nc.declare_dram_parameter("tri", [128, 128], BF16, isOutput=False)
    out = nc.declare_dram_parameter("out", [BS, DIM], BF16, isOutput=True)

    with TileContext(nc) as tc:
        with (
            tc.tile_pool(name="const", bufs=1) as const,
            tc.tile_pool(name="xtp", bufs=3) as xtp,
            tc.tile_pool(name="ropep", bufs=2) as ropep,
            tc.tile_pool(name="ptp", bufs=8) as ptp,
            tc.tile_pool(name="atp", bufs=3) as atp,
            tc.tile_pool(name="accp", bufs=2) as accp,
            tc.tile_pool(name="recp", bufs=2) as recp,
            tc.tile_pool(name="osb", bufs=3) as osb,
            tc.tile_pool(name="psA", bufs=4, space="PSUM") as psA,
            tc.tile_pool(name="psB", bufs=4, space="PSUM") as psB,
        ):
            # ---- resident constants / weights -------------------------------
            # wq/wk/wv are DMA'd per k-slice inside the j==0 loop so the
            # first matmuls start as soon as their slice lands.
            wq_sb = const.tile([128, KT, HPC * HD], BF16, tag="wq")
            wk_sb = const.tile([128, KT, HD], BF16, tag="wk")
            wv_sb = const.tile([128, KT, HD], BF16, tag="wv")
            cos_sb = const.tile([64, SEQ], BF16, tag="cos")
            nc.sync.dma_start(cos_sb[:], cosT[:])
            sin_sb = const.tile([64, SEQ], BF16, tag="sin")
            nc.sync.dma_start(sin_sb[:], sinT[:])
            tri_sb = const.tile([128, 128], BF16, tag="tri")
            nc.sync.dma_start(tri_sb[:], tri[:])
            # wo is first needed in the attention phase; loaded there.
            wo_sb = const.tile([128, HPC, DIM], BF16, tag="wo")

            ones_sb = const.tile([128, 1], BF16, tag="ones")
            nc.gpsimd.memset(ones_sb[:], 1.0)
            ident = const.tile([128, 128], BF16, tag="ident")
            make_identity(nc, ident[:])

            # Preload the exp table so the first attention exp doesn't pay
            # the ACT_TABLE_LOAD, and run warm-up matmuls on ident during the
            # initial DMA window so HAM unthrottles before the real work.
            scr = const.tile([128, 1], BF16, tag="scr")
            nc.scalar.activation(scr[:], ones_sb[:], AF.Exp)
            warm = psA.tile([128, 512], F32, tag="psA", name="warm")
            for _ in range(130):
                nc.tensor.matmul(warm[:, 0:128], lhsT=ident[:], rhs=ident[:],
                                 start=True, stop=True)

            # ---- resident activations ---------------------------------------
            qT_sb = const.tile([128, HPC, BS], BF16, tag="qT")    # per-head Q^T
            kT_sb = const.tile([128, BS], BF16, tag="kT")         # K^T (d, t)
            v_sb = const.tile([128, BS // 128, HD], BF16, tag="v")  # V (t, d) tiles
            # attention output lives per-block in the atp pool (2 blocks live)

            def rope_copy(psum, dst, soff):
                """psum [128,512] (evens-first layout) -> rotated bf16 dst."""
                te = ropep.tile([64, 512], BF16, tag="ropetmpe")
                to = ropep.tile([64, 512], BF16, tag="ropetmpo")
                nc.scalar.copy(te[:], psum[0:64])
                nc.vector.tensor_copy(to[:], psum[64:128])
                cs = cos_sb[:, soff:soff + 512]
                sn = sin_sb[:, soff:soff + 512]
                te = te[:]
                to = to[:]
                t1 = ropep.tile([64, 512], BF16, tag="t1")
                t2 = ropep.tile([64, 512], BF16, tag="t2")
                nc.vector.tensor_tensor(t1[:], te, cs, ALU.mult)
                nc.vector.tensor_tensor(t2[:], to, sn, ALU.mult)
                nc.vector.tensor_tensor(dst[0:64], t1[:], t2[:], ALU.subtract)
                t3 = ropep.tile([64, 512], BF16, tag="t1")
                t4 = ropep.tile([64, 512], BF16, tag="t2")
                nc.vector.tensor_tensor(t3[:], te, sn, ALU.mult)
                nc.vector.tensor_tensor(t4[:], to, cs, ALU.mult)
                nc.vector.tensor_tensor(dst[64:128], t3[:], t4[:], ALU.add)

            # ================= Phase 1: QKV projection =======================
            # single pass over xT per s-chunk: 4 Q accumulators in psA,
            # K/V accumulators in psB.
            for j in range(NJ):
                soff = (j % SBLK) * 512      # within-batch s offset
                js = slice(j * 512, (j + 1) * 512)
                qps = [psA.tile([128, 512], F32, tag="psA", name=f"qps{j}_{c}") for c in range(HPC)]
                kp = psB.tile([128, 512], F32, tag="psB", name=f"kp{j}")
                vp = psB.tile([128, 512], F32, tag="psB", name=f"vp{j}")
                for kc in range(KT // 4):
                    xt = xtp.tile([128, 4, 512], BF16, tag="xt")
                    nc.sync.dma_start(
                        xt[:],
                        xT[kc * 512:(kc + 1) * 512, js].rearrange("(a p) m -> p a m", p=128))
                    if j == 0:
                        for k4 in range(4):
                            ks = slice((kc * 4 + k4) * 128, (kc * 4 + k4 + 1) * 128)
                            nc.sync.dma_start(wq_sb[:, kc * 4 + k4, :], wqT[ks, :])
                            nc.sync.dma_start(wk_sb[:, kc * 4 + k4, :], wkT[ks, :])
                            nc.sync.dma_start(wv_sb[:, kc * 4 + k4, :], wvT[ks, :])
                    for k4 in range(4):
                        k = kc * 4 + k4
                        for c in range(HPC):
                            nc.tensor.matmul(
                                qps[c][:], lhsT=wq_sb[:, k, c * 128:(c + 1) * 128],
                                rhs=xt[:, k4, :], start=(k == 0), stop=(k == KT - 1))
                        nc.tensor.matmul(kp[:], lhsT=wk_sb[:, k, :], rhs=xt[:, k4, :],
                                         start=(k == 0), stop=(k == KT - 1))
                        nc.tensor.matmul(vp[:], lhsT=wv_sb[:, k, :], rhs=xt[:, k4, :],
                                         start=(k == 0), stop=(k == KT - 1))
                # K/V first: attention needs them (and their PSUM slots) at the
                # phase boundary before any Q-rope results.
                rope_copy(kp, kT_sb[:, js], soff)
                # V^T chunk -> natural-layout V tiles via PE transpose.
                # Last chunk's copies go on DVE so the ScalarE queue is clear
                # for the first attention exp right at the phase boundary.
                last = j == NJ - 1
                vtmp = ropep.tile([128, 512], BF16, tag="vtmp")
                if last:
                    nc.vector.tensor_copy(vtmp[:], vp[:])
                else:
                    nc.scalar.copy(vtmp[:], vp[:])
                for sub in range(4):
                    tt = j * 4 + sub
                    pvt = psB.tile([128, 512], BF16, tag="psB", name=f"pvt{j}_{sub}")
                    with nc.allow_low_precision(reason="pure transpose, no accumulation"):
                        nc.tensor.transpose(
                            pvt[:, 0:128], vtmp[:, sub * 128:(sub + 1) * 128], ident[:])
                    if last:
                        nc.vector.tensor_copy(v_sb[:, tt, :], pvt[:, 0:128])
                    else:
                        nc.scalar.copy(v_sb[:, tt, :], pvt[:, 0:128])
                for c in range(HPC):
                    rope_copy(qps[c], qT_sb[:, c, js], soff)
                if j == 4:
                    # mid-phase: DMA bandwidth has headroom here and wo is
                    # needed right after the phase boundary.
                    nc.sync.dma_start(wo_sb[:], woT.rearrange("(a p) m -> p a m", p=128))

            # ================= Phase 2+3: attention + out-proj ===============
            # Out-proj of the previous block is interleaved at attention-tile
            # granularity so the PE never starves while ScalarE runs exp; the
            # den/recip chain of each head is deferred by one head so its
            # den-matmul never blocks the PE FIFO on the DVE accumulation.
            def outproj_group(b, s_lo, s_w, at_blk, g):
                st, n = g // 8, g % 8
                s0 = b * SEQ + s_lo + st * 128
                po = psA.tile([128, 512], F32, tag="psA", name=f"po{b}_{s_lo}_{g}")
                for dt in range(HPC):
                    nc.tensor.matmul(
                        po[:], lhsT=at_blk[:, dt, st * 128:(st + 1) * 128],
                        rhs=wo_sb[:, dt, n * 512:(n + 1) * 512],
                        start=(dt == 0), stop=(dt == HPC - 1))
                ob = osb.tile([128, 512], BF16, tag="ob")
                if n % 2 == 0:
                    nc.scalar.copy(ob[:], po[:])
                else:
                    nc.vector.tensor_copy(ob[:], po[:])
                nc.sync.dma_start(out[s0:s0 + 128, n * 512:(n + 1) * 512], ob[:])

            def den_chain(h, s_w, ppv, ptsum, at_blk):
                pden = psB.tile([1, 512], F32, tag="psB", name=f"pden_{h}")
                nc.tensor.matmul(pden[:, 0:s_w], lhsT=ones_sb[:],
                                 rhs=ptsum[:, 0:s_w], start=True, stop=True)
                recf = recp.tile([1, 512], F32, tag="recf")
                nc.vector.reciprocal_approx_fast(recf[:, 0:s_w], pden[:, 0:s_w])
                rec = recp.tile([1, 512], BF16, tag="rec")
                nc.vector.tensor_copy(rec[:, 0:s_w], recf[:, 0:s_w])
                rep = recp.tile([128, 512], BF16, tag="rep")
                nc.gpsimd.partition_broadcast(rep[:, 0:s_w], rec[:, 0:s_w])
                nc.vector.tensor_tensor(at_blk[:, h, 0:s_w],
                                        ppv[:, 0:s_w], rep[:, 0:s_w], ALU.mult)

            # (batch, s_lo, s_width); within a batch s ranges must ascend.
            # The final block is split in two halves so half the last
            # out-proj overlaps the second half's attention.
            blocks = [(0, 0, 512), (0, 512, 512), (0, 1024, 512), (0, 1536, 512),
                      (1, 512, 512), (1, 1024, 512), (1, 1536, 512),
                      (1, 0, 256), (1, 256, 256)]
            prev = None
            for bi, (b, s_lo, s_w) in enumerate(blocks):
                # First block overlaps the tail of the phase-1 rope work on
                # DVE; push its accumulation to the idle GpSimd engine.
                acc_eng = nc.gpsimd if bi == 0 else nc.vector
                sg = b * SEQ + s_lo
                nt = (s_lo + s_w) // 128             # causal t-tiles
                ntile = HPC * nt
                ngrp_prev = (prev[2] // 128) * 8 if prev is not None else 0
                at_blk = atp.tile([128, HPC, 512], BF16, tag="attnT",
                                  name=f"attnT{sg}")
                emitted = 0
                tidx = 0
                pend = None
                for h in range(HPC):
                    ppv = psA.tile([128, 512], F32, tag="psA", name=f"ppv{sg}{h}")
                    ptsum = accp.tile([128, 512], BF16, tag="ptsum",
                                      name=f"ptsum{sg}{h}")
                    for ti in range(nt):
                        tg = b * SEQ + ti * 128
                        w0 = max(0, 128 * ti - s_lo)
                        diag = 128 * ti >= s_lo
                        psc = psB.tile([128, 512], F32, tag="psB",
                                       name=f"psc{sg}{h}{ti}")
                        nc.tensor.matmul(psc[:, w0:s_w],
                                         lhsT=kT_sb[:, tg:tg + 128],
                                         rhs=qT_sb[:, h, sg + w0:sg + s_w],
                                         start=True, stop=not diag)
                        if diag:                     # additive causal mask via PE
                            nc.tensor.matmul(psc[:, w0:w0 + 128], lhsT=ident[:],
                                             rhs=tri_sb[:],
                                             start=False, stop=True)
                        if ti == 0:
                            pt = ptsum               # exp seeds the running sum
                        else:
                            pt = ptp.tile([128, 512], BF16, tag="pt")
                        nc.scalar.activation(pt[:, w0:s_w], psc[:, w0:s_w], AF.Exp)
                        if ti > 0:
                            acc_eng.tensor_tensor(ptsum[:, w0:s_w],
                                                  ptsum[:, w0:s_w],
                                                  pt[:, w0:s_w], ALU.add)
                        nc.tensor.matmul(ppv[:, w0:s_w],
                                         lhsT=v_sb[:, tg // 128, :],
                                         rhs=pt[:, w0:s_w], start=(ti == 0),
                                         stop=(ti == nt - 1))
                        tidx += 1
                        if prev is not None:
                            want = tidx * ngrp_prev // ntile
                            while emitted < want:
                                outproj_group(*prev, emitted)
                                emitted += 1
                    if pend is not None:
                        den_chain(*pend)
                    pend = (h, s_w, ppv, ptsum, at_blk)
                if prev is not None:
                    while emitted < ngrp_prev:
                        outproj_group(*prev, emitted)
                        emitted += 1
                den_chain(*pend)
                prev = (b, s_lo, s_w, at_blk)
            for g in range((prev[2] // 128) * 8):
                outproj_group(*prev, g)
    nc.finalize()
    return nc


_GRAPH = None


def _get_graph():
    global _GRAPH
    if _GRAPH is None:
        _GRAPH = build_graph()
    return _GRAPH


def prepare_in_maps(x, wq, wk, wv, wo, freqs_cos, freqs_sin, mask, start_pos=0):
    x = np.asarray(x, np.float32)
    wq = np.asarray(wq, np.float32)
    wk = np.asarray(wk, np.float32)
    wv = np.asarray(wv, np.float32)
    wo = np.asarray(wo, np.float32)
    fc = np.asarray(freqs_cos, np.float32)
    fs = np.asarray(freqs_sin, np.float32)

    # evens-first pair permutation (interleaved rope -> rotate-half form)
    perm = np.concatenate([np.arange(0, HD, 2), np.arange(1, HD, 2)])

    def permute_heads(w):
        wr = w.reshape(-1, HD, DIM)[:, perm, :]
        return wr.reshape(-1, DIM)

    wq_p = permute_heads(wq) * (1.0 / math.sqrt(HD))
    wk_p = permute_heads(wk)

    xT = np.ascontiguousarray(x.reshape(BS, DIM).T).astype(NPBF16)
    cosT = np.ascontiguousarray(fc.T).astype(NPBF16)
    sinT = np.ascontiguousarray(fs.T).astype(NPBF16)
    # additive causal triangle for the in-tile diagonal: tri[t, c] = 0 if
    # c >= t else -1e9 (c = column within the 128-wide diagonal strip)
    tt, cc = np.meshgrid(np.arange(128), np.arange(128), indexing="ij")
    tri = np.where(cc >= tt, 0.0, -1e9).astype(NPBF16)

    in_maps = []
    for c in range(NCORES):
        qs = slice(c * HPC * HD, (c + 1) * HPC * HD)
        ks = slice(c * HD, (c + 1) * HD)
        in_maps.append({
            "xT": xT,
            "wqT": np.ascontiguousarray(wq_p[qs, :].T).astype(NPBF16),
            "wkT": np.ascontiguousarray(wk_p[ks, :].T).astype(NPBF16),
            "wvT": np.ascontiguousarray(wv[ks, :].T).astype(NPBF16),
            "woT": np.ascontiguousarray(wo[:, qs].T).astype(NPBF16),
            "cosT": cosT,
            "sinT": sinT,
            "tri": tri,
        })
    return in_maps


def combine_results(results):
    acc = results[0]["out"].astype(np.float64)
    for c in range(1, NCORES):
        acc = acc + results[c]["out"]
    return acc.astype(np.float32).reshape(BSZ, SEQ, DIM)


def run_spmd(in_maps, **kw):
    nc = _get_graph()
    return run_bass_kernel_spmd(nc, in_maps, list(range(NCORES)), **kw)


def kernel(x, wq, wk, wv, wo, freqs_cos, freqs_sin, mask, start_pos=0, **_):
    in_maps = prepare_in_maps(x, wq, wk, wv, wo, freqs_cos, freqs_sin, mask)
    res = run_spmd(in_maps)
    return combine_results(res.results)
